# revision 16
# baseline (speedup 1.0000x reference)
"""Conv2d 3x3 (stride 1, pad 1) as implicit GEMM on 8 Trainium2 NeuronCores.

Problem: x[32,128,56,56] f32, weights[128,128,3,3] f32, bias[128] f32
         -> out[32,128,56,56] f32.

Sharding: data-parallel over batch — 4 images per core, weights/bias
replicated on every core.

Per-core kernel design (v2 — bf16 operands, 2D windows):
  - channels (128) live on the SBUF partition dim; x and weights are
    converted to bf16 on the host (conv rel-err ~2e-3, well inside the
    gate) which halves input DMA bytes and — critically — enables the
    PE's automatic Fast Weight Load path (disabled for fp32/fp32r), so
    the per-matmul LDWEIGHTS fully hides behind the 448-col stream and
    the matmul cadence drops from ~210ns to ~stream time.
  - the host pre-pads each image into a flat row layout
      [margin 58 | 56 rows x (56 data + 1 zero) | margin 58]
    so every conv tap (kh,kw) is a shifted window of one flat buffer.
  - matmul rhs uses a 2D access pattern [8 rows x 56 cols, row stride
    57]: each PSUM group is exactly 448 interior columns (no pad-col
    compute) and maps 1:1 onto the contiguous output slice, so the
    bias-fusing PSUM->SBUF eviction on the scalar engine is a plain
    copy.
  - output DMA kicks are issued from the Scalar queue right after each
    group's eviction (Activation is a hardware-DGE engine): no
    cross-engine semaphore and the input ring on the Sync queue stays
    pure-input, so in-order queue service naturally prioritizes the
    early tiles — no manual gating needed.
  - input kicks are ordered [w taps0-2, x0 head, w rest, x0 rest,
    x1..x3] so the first group's operands land as early as possible;
    bf16 warmup matmuls on a zero scratch ramp the PE clock (HAM)
    during the DMA wait.
"""

import numpy as np

N_TOTAL = 32
N_CORES = 8
N_PER_CORE = N_TOTAL // N_CORES
C = 128
H = W = 56
HW = H * W            # 3136
WP = W + 1            # 57  padded row width (shared pad col)
L = H * WP            # 3192 flat padded length
MARGIN = WP + 1       # 58  covers worst tap offset
TILE_W = MARGIN + L + MARGIN  # 3308
ROWS_PER_G = 8
GW = ROWS_PER_G * WP  # 456 flat span of one group window
GI = ROWS_PER_G * W   # 448 interior columns per group
N_GROUPS = H // ROWS_PER_G  # 7
N_WARMUP = 9
# x0 arrives in 3 chunks so group 1+ never waits on the whole image;
# later images land as single transfers (the in-order ring gives the
# earlier chunks priority automatically).
X0_BOUNDS = [0, MARGIN + GW + MARGIN, MARGIN + 3 * GW + MARGIN, TILE_W]

_CACHE = {}


def _build_nc():
    import concourse.mybir as mybir
    import concourse.tile as tile
    from concourse import bacc

    f32 = mybir.dt.float32
    bf16 = mybir.dt.bfloat16
    af = mybir.ActivationFunctionType

    nc = bacc.Bacc("TRN2", target_bir_lowering=False, debug=False)

    x_d = nc.dram_tensor("x", [N_PER_CORE, C, TILE_W], bf16, kind="ExternalInput")
    w_d = nc.dram_tensor("w", [C, 9 * C], bf16, kind="ExternalInput")
    b_d = nc.dram_tensor("b", [C, 1], f32, kind="ExternalInput")
    y_d = nc.dram_tensor("y", [N_PER_CORE, C, HW], f32, kind="ExternalOutput")

    with tile.TileContext(nc) as tc:
        with (
            tc.tile_pool(name="const", bufs=1) as cpool,
            tc.tile_pool(name="xbuf", bufs=1) as xpool,
            tc.tile_pool(name="obuf", bufs=2) as opool,
            tc.tile_pool(name="psum", bufs=4, space="PSUM") as ppool,
            tc.tile_pool(name="warm", bufs=2, space="PSUM") as wpool,
        ):
            # PE warm-up on a zero scratch: the HAM clock gate needs ~3us
            # of PE activity before full speed; sized so the ramp
            # completes about when the first input chunks land.
            zsc = cpool.tile([C, 256], bf16, tag="zsc")
            nc.gpsimd.memset(zsc[:], 0.0)
            for k in range(N_WARMUP):
                wm = wpool.tile([C, 256], f32, tag="wm")
                nc.tensor.matmul(
                    wm[:], zsc[:, 0:C], zsc[:], start=True, stop=True
                )

            # All inputs ride the Sync queue's DMA ring, in priority
            # order — the ring is serviced in order, and DMA is packet-
            # bound (one ~80-250ns packet per partition row, cost flat
            # below ~2KB), so w goes as ONE kick (128 packets instead of
            # 3x128 for split chunks) and is fully resident before
            # group 0 ends; x0 stays in 3 chunks for finer deps.
            wt = cpool.tile([C, 9 * C], bf16, tag="wt")
            xts = []
            for n in range(N_PER_CORE):
                xt = xpool.tile([C, TILE_W], bf16, tag=f"x{n}")
                xts.append(xt)
            # The very first chunks arrive as 64-partition halves (half
            # the DMA packets) so group 0's first tap can start ~1us
            # earlier as two K=64 matmuls accumulating into one PSUM
            # group; everything after uses the full 128 partitions.
            hx = X0_BOUNDS[1]
            nc.sync.dma_start(
                out=wt[0:64, 0 : 6 * C], in_=w_d[0:64, 0 : 6 * C]
            )
            nc.sync.dma_start(out=xts[0][0:64, 0:hx], in_=x_d[0][0:64, 0:hx])
            nc.sync.dma_start(
                out=wt[64:128, 0 : 6 * C], in_=w_d[64:128, 0 : 6 * C]
            )
            nc.sync.dma_start(
                out=xts[0][64:128, 0:hx], in_=x_d[0][64:128, 0:hx]
            )
            nc.sync.dma_start(out=wt[:, 6 * C : 9 * C], in_=w_d[:, 6 * C : 9 * C])
            for a, b in zip(X0_BOUNDS[1:], X0_BOUNDS[2:]):
                nc.sync.dma_start(out=xts[0][:, a:b], in_=x_d[0][:, a:b])
            for n in range(1, N_PER_CORE):
                nc.sync.dma_start(out=xts[n][:], in_=x_d[n][:])

            # Bias rides the Scalar queue's ring (also hardware-DGE);
            # it's one packet per partition and lands long before the
            # first eviction.
            bt = cpool.tile([C, 1], f32, tag="bt")
            nc.scalar.dma_start(out=bt[:], in_=b_d[:])

            for n in range(N_PER_CORE):
                ot = opool.tile([C, HW], f32, tag="out")
                for g in range(N_GROUPS):
                    ps = ppool.tile([C, GI], f32, tag="ps")
                    for t in range(9):
                        kh, kw = divmod(t, 3)
                        base = MARGIN + g * GW + (kh - 1) * WP + (kw - 1)
                        rhs = (
                            xts[n][:, base : base + GW]
                            .rearrange("p (r c) -> p r c", c=WP)[:, :, 0:W]
                        )
                        if n == 0 and g == 0 and t == 0:
                            # first tap as two K=64 halves so it only
                            # waits on the 64-partition head transfers
                            for p0, p1, st in ((0, 64, True), (64, 128, False)):
                                rhs_h = (
                                    xts[0][p0:p1, base : base + GW]
                                    .rearrange("p (r c) -> p r c", c=WP)[:, :, 0:W]
                                )
                                nc.tensor.matmul(
                                    ps[:], wt[p0:p1, 0:C], rhs_h,
                                    start=st, stop=False,
                                )
                            continue
                        lhsT = wt[:, t * C : (t + 1) * C]
                        nc.tensor.matmul(
                            ps[:], lhsT, rhs, start=(t == 0), stop=(t == 8)
                        )
                    dst = ot[:, g * GI : (g + 1) * GI]
                    nc.scalar.activation(dst, ps[:], af.Identity, bias=bt[:])
                    nc.scalar.dma_start(
                        out=y_d[n][:, g * GI : (g + 1) * GI], in_=dst
                    )

    nc.compile()
    return nc


def _get_nc():
    if "nc" not in _CACHE:
        _CACHE["nc"] = _build_nc()
    return _CACHE["nc"]


def _prep_inputs(x, weights, bias):
    import ml_dtypes

    bf16 = ml_dtypes.bfloat16
    x = np.asarray(x, dtype=np.float32).reshape(N_TOTAL, C, H, W)
    xp = np.zeros((N_TOTAL, C, TILE_W), dtype=bf16)
    # interior: rows of [56 data | 0], flat at offset MARGIN
    v = xp[:, :, MARGIN : MARGIN + L].reshape(N_TOTAL, C, H, WP)
    v[:, :, :, 0:W] = x.astype(bf16)
    # weights [co, ci, kh, kw] -> [ci, (kh kw), co] so each tap slice is a
    # contiguous [K=ci, M=co] lhsT tile
    w = np.asarray(weights, dtype=np.float32)
    w = np.ascontiguousarray(
        np.transpose(w, (1, 2, 3, 0)).reshape(C, 9 * C)
    ).astype(bf16)
    b = np.ascontiguousarray(np.asarray(bias, dtype=np.float32).reshape(C, 1))
    return xp, w, b


def kernel(x, weights, bias, _trace=False):
    from concourse.bass_utils import run_bass_kernel_spmd

    nc = _get_nc()
    xp, w, b = _prep_inputs(x, weights, bias)
    in_maps = [
        {"x": xp[i * N_PER_CORE : (i + 1) * N_PER_CORE], "w": w, "b": b}
        for i in range(N_CORES)
    ]
    res = run_bass_kernel_spmd(
        nc, in_maps, core_ids=list(range(N_CORES)), trace=_trace
    )
    y = np.concatenate([r["y"] for r in res.results], axis=0)
    y = y.reshape(N_TOTAL, C, H, W)
    if _trace:
        return y, res
    return y


# revision 19
# speedup vs baseline: 1.0344x; 1.0344x over previous
"""Conv2d 3x3 (stride 1, pad 1) as implicit GEMM on 8 Trainium2 NeuronCores.

Problem: x[32,128,56,56] f32, weights[128,128,3,3] f32, bias[128] f32
         -> out[32,128,56,56] f32.

Sharding: data-parallel over batch — 4 images per core, weights/bias
replicated on every core.

Per-core kernel design (v2 — bf16 operands, 2D windows):
  - channels (128) live on the SBUF partition dim; x and weights are
    converted to bf16 on the host (conv rel-err ~2e-3, well inside the
    gate) which halves input DMA bytes and — critically — enables the
    PE's automatic Fast Weight Load path (disabled for fp32/fp32r), so
    the per-matmul LDWEIGHTS fully hides behind the 448-col stream and
    the matmul cadence drops from ~210ns to ~stream time.
  - the host pre-pads each image into a flat row layout
      [margin 58 | 56 rows x (56 data + 1 zero) | margin 58]
    so every conv tap (kh,kw) is a shifted window of one flat buffer.
  - matmul rhs uses a 2D access pattern [8 rows x 56 cols, row stride
    57]: each PSUM group is exactly 448 interior columns (no pad-col
    compute) and maps 1:1 onto the contiguous output slice, so the
    bias-fusing PSUM->SBUF eviction on the scalar engine is a plain
    copy.
  - output DMA kicks are issued from the Scalar queue right after each
    group's eviction (Activation is a hardware-DGE engine): no
    cross-engine semaphore and the input ring on the Sync queue stays
    pure-input, so in-order queue service naturally prioritizes the
    early tiles — no manual gating needed.
  - input kicks are ordered [w taps0-2, x0 head, w rest, x0 rest,
    x1..x3] so the first group's operands land as early as possible;
    bf16 warmup matmuls on a zero scratch ramp the PE clock (HAM)
    during the DMA wait.
"""

import numpy as np

N_TOTAL = 32
N_CORES = 8
N_PER_CORE = N_TOTAL // N_CORES
C = 128
H = W = 56
HW = H * W            # 3136
WP = W + 1            # 57  padded row width (shared pad col)
L = H * WP            # 3192 flat padded length
MARGIN = WP + 1       # 58  covers worst tap offset
TILE_W = MARGIN + L + MARGIN  # 3308
ROWS_PER_G = 8
GW = ROWS_PER_G * WP  # 456 flat span of one group window
GI = ROWS_PER_G * W   # 448 interior columns per group
N_GROUPS = H // ROWS_PER_G  # 7
N_WARMUP = 14
# x0 arrives in 3 chunks so group 1+ never waits on the whole image;
# later images land as single transfers (the in-order ring gives the
# earlier chunks priority automatically).
X0_BOUNDS = [0, MARGIN + GW + MARGIN, MARGIN + 3 * GW + MARGIN, TILE_W]

_CACHE = {}


def _build_nc():
    import concourse.mybir as mybir
    import concourse.tile as tile
    from concourse import bacc

    f32 = mybir.dt.float32
    bf16 = mybir.dt.bfloat16
    af = mybir.ActivationFunctionType

    nc = bacc.Bacc("TRN2", target_bir_lowering=False, debug=False)

    x_d = nc.dram_tensor("x", [N_PER_CORE, C, TILE_W], bf16, kind="ExternalInput")
    w_d = nc.dram_tensor("w", [C, 9 * C], bf16, kind="ExternalInput")
    b_d = nc.dram_tensor("b", [C, 1], f32, kind="ExternalInput")
    y_d = nc.dram_tensor("y", [N_PER_CORE, C, HW], f32, kind="ExternalOutput")

    with tile.TileContext(nc) as tc:
        with (
            tc.tile_pool(name="const", bufs=1) as cpool,
            tc.tile_pool(name="xbuf", bufs=1) as xpool,
            tc.tile_pool(name="obuf", bufs=2) as opool,
            tc.tile_pool(name="psum", bufs=4, space="PSUM") as ppool,
            tc.tile_pool(name="warm", bufs=2, space="PSUM") as wpool,
        ):
            # PE warm-up on a zero scratch: the HAM clock gate needs ~3us
            # of PE activity before full speed; sized so the ramp
            # completes about when the first input chunks land.
            zsc = cpool.tile([C, 256], bf16, tag="zsc")
            nc.gpsimd.memset(zsc[:], 0.0)
            for k in range(N_WARMUP):
                wm = wpool.tile([C, 256], f32, tag="wm")
                nc.tensor.matmul(
                    wm[:], zsc[:, 0:C], zsc[:], start=True, stop=True
                )

            # All inputs ride the Sync queue's DMA ring, in priority
            # order — the ring is serviced in order, and DMA is packet-
            # bound (one ~80-250ns packet per partition row, cost flat
            # below ~2KB), so w goes as ONE kick (128 packets instead of
            # 3x128 for split chunks) and is fully resident before
            # group 0 ends; x0 stays in 3 chunks for finer deps.
            wt = cpool.tile([C, 9 * C], bf16, tag="wt")
            xts = []
            for n in range(N_PER_CORE):
                xt = xpool.tile([C, TILE_W], bf16, tag=f"x{n}")
                xts.append(xt)
            nc.sync.dma_start(out=wt[:, 0 : 6 * C], in_=w_d[:, 0 : 6 * C])
            nc.sync.dma_start(
                out=xts[0][:, X0_BOUNDS[0] : X0_BOUNDS[1]],
                in_=x_d[0][:, X0_BOUNDS[0] : X0_BOUNDS[1]],
            )
            nc.sync.dma_start(out=wt[:, 6 * C : 9 * C], in_=w_d[:, 6 * C : 9 * C])
            for a, b in zip(X0_BOUNDS[1:], X0_BOUNDS[2:]):
                nc.sync.dma_start(out=xts[0][:, a:b], in_=x_d[0][:, a:b])
            for n in range(1, N_PER_CORE):
                nc.sync.dma_start(out=xts[n][:], in_=x_d[n][:])

            # Bias rides the Scalar queue's ring (also hardware-DGE);
            # it's one packet per partition and lands long before the
            # first eviction.
            bt = cpool.tile([C, 1], f32, tag="bt")
            nc.scalar.dma_start(out=bt[:], in_=b_d[:])

            for n in range(N_PER_CORE):
                ot = opool.tile([C, HW], f32, tag="out")
                for g in range(N_GROUPS):
                    ps = ppool.tile([C, GI], f32, tag="ps")
                    for t in range(9):
                        kh, kw = divmod(t, 3)
                        base = MARGIN + g * GW + (kh - 1) * WP + (kw - 1)
                        rhs = (
                            xts[n][:, base : base + GW]
                            .rearrange("p (r c) -> p r c", c=WP)[:, :, 0:W]
                        )
                        lhsT = wt[:, t * C : (t + 1) * C]
                        nc.tensor.matmul(
                            ps[:], lhsT, rhs, start=(t == 0), stop=(t == 8)
                        )
                    dst = ot[:, g * GI : (g + 1) * GI]
                    nc.scalar.activation(dst, ps[:], af.Identity, bias=bt[:])
                    nc.scalar.dma_start(
                        out=y_d[n][:, g * GI : (g + 1) * GI], in_=dst
                    )

    nc.compile()
    return nc


def _get_nc():
    if "nc" not in _CACHE:
        _CACHE["nc"] = _build_nc()
    return _CACHE["nc"]


def _prep_inputs(x, weights, bias):
    import ml_dtypes

    bf16 = ml_dtypes.bfloat16
    x = np.asarray(x, dtype=np.float32).reshape(N_TOTAL, C, H, W)
    xp = np.zeros((N_TOTAL, C, TILE_W), dtype=bf16)
    # interior: rows of [56 data | 0], flat at offset MARGIN
    v = xp[:, :, MARGIN : MARGIN + L].reshape(N_TOTAL, C, H, WP)
    v[:, :, :, 0:W] = x.astype(bf16)
    # weights [co, ci, kh, kw] -> [ci, (kh kw), co] so each tap slice is a
    # contiguous [K=ci, M=co] lhsT tile
    w = np.asarray(weights, dtype=np.float32)
    w = np.ascontiguousarray(
        np.transpose(w, (1, 2, 3, 0)).reshape(C, 9 * C)
    ).astype(bf16)
    b = np.ascontiguousarray(np.asarray(bias, dtype=np.float32).reshape(C, 1))
    return xp, w, b


def kernel(x, weights, bias, _trace=False):
    from concourse.bass_utils import run_bass_kernel_spmd

    nc = _get_nc()
    xp, w, b = _prep_inputs(x, weights, bias)
    in_maps = [
        {"x": xp[i * N_PER_CORE : (i + 1) * N_PER_CORE], "w": w, "b": b}
        for i in range(N_CORES)
    ]
    res = run_bass_kernel_spmd(
        nc, in_maps, core_ids=list(range(N_CORES)), trace=_trace
    )
    y = np.concatenate([r["y"] for r in res.results], axis=0)
    y = y.reshape(N_TOTAL, C, H, W)
    if _trace:
        return y, res
    return y
